# revision 1
# baseline (speedup 1.0000x reference)
"""Trainium2 Bass kernel for nn_DIDAModule (dense_cnn).

Math: the per-sample "dynamic" depthwise kernels are affine in the channel
gate g:  kern1 = g*A1 + B1  with  A1 = wk*wck, B1 = bk*wck + bck  (5x5) and
A2 = wk2*wck2, B2 = bk2*wck2 + bck2 (3x3, dilation 2).  Per-channel scaling
commutes with the (channel-shared) depthwise convs, so

    o1 = conv_A1(g*f) + conv_B1(f)      o2 = conv_A2(g*f) + conv_B2(f)
    y  = [W_fuse @ o1 + b_fuse ; W_fuse @ o2 + b_fuse]

The four static depthwise convs are done as banded spatial matmuls on the
Tensor engine: spatial-major layout (via DMA-xbar transposes in bf16), flat
128-pixel blocks, 7 phase classes (128 mod 56 = 16, period 7), and 3 band
position matrices (prev/self/next block) per phase per kernel, built host
side exactly from the conv geometry (including row-edge handling).

Sharding: data-parallel over batch N across the 8 cores (4 samples each),
weights replicated.
"""

import numpy as np

# ---------------------------------------------------------------- dims
N, C, H, W = 32, 512, 56, 56
CM, K1, K2, P2 = 128, 5, 3, 256
HW = H * W            # 3136
SP = 3200             # padded spatial: 25 blocks of 128
NB = 25
PH = 7                # phase classes
NCORES = 8
NPC = N // NCORES     # samples per core
EW = 392              # conv1 free-block width (8 per sample)
FW = 400              # fuse free-block width (8 per padded map)

_CACHE = {}


# ---------------------------------------------------------------- host prep
def _build_T(K2d, dil):
    """Banded conv matrices T[phase, pos, k_in, m_out] for flat 128-blocks."""
    kh = K2d.shape[0]
    r = (kh - 1) // 2 * dil
    T = np.zeros((PH, 3, 128, 128), np.float32)
    for p in range(PH):
        bref = 7 + p              # interior reference block of this phase
        for pos, d in enumerate((-1, 0, 1)):
            for m in range(128):
                s_out = bref * 128 + m
                ro, wo = divmod(s_out, W)
                for k in range(128):
                    s_in = (bref + d) * 128 + k
                    ri, wi = divmod(s_in, W)
                    di, dj = ri - ro, wi - wo
                    if (abs(di) <= r and abs(dj) <= r
                            and di % dil == 0 and dj % dil == 0):
                        T[p, pos, k, m] = K2d[di // dil + (kh - 1) // 2,
                                              dj // dil + (kh - 1) // 2]
    return T


def _host_consts(inp):
    import ml_dtypes
    bf16 = ml_dtypes.bfloat16
    W_conv = np.asarray(inp["W_conv"], np.float32)     # [CM, C]
    W_fuse = np.asarray(inp["W_fuse"], np.float32)     # [P2, CM]
    A1 = (np.asarray(inp["wk"]) * float(inp["wck"])).reshape(K1, K1)
    B1 = (np.asarray(inp["bk"]) * float(inp["wck"]) + float(inp["bck"])).reshape(K1, K1)
    A2 = (np.asarray(inp["wk2"]) * float(inp["wck2"])).reshape(K2, K2)
    B2 = (np.asarray(inp["bk2"]) * float(inp["wck2"]) + float(inp["bck2"])).reshape(K2, K2)
    # T layout: [k_in(128part), kern(4), ph(7), pos(3), m_out(128)]
    T = np.stack([_build_T(A1.astype(np.float32), 1),
                  _build_T(B1.astype(np.float32), 1),
                  _build_T(A2.astype(np.float32), 2),
                  _build_T(B2.astype(np.float32), 2)])      # [4,7,3,128,128]
    T_h = np.ascontiguousarray(T.transpose(3, 0, 1, 2, 4)).reshape(128, 84 * 128)
    # conv1 lhsT chunks: [c_local(128part), kc(4), cm(128)]
    wconvT_h = np.ascontiguousarray(
        W_conv.T.reshape(4, 128, CM).transpose(1, 0, 2)).reshape(128, 4 * CM)
    # fuse lhsT chunks: [c(128part), chunk(2), o_local(128)]
    wfuseT_h = np.ascontiguousarray(
        W_fuse.T.reshape(CM, 2, 128)).reshape(CM, 256)
    return {
        "wconvT": wconvT_h.astype(np.float32),
        "bconv": np.asarray(inp["b_conv"], np.float32).reshape(CM, 1),
        "Tmat": T_h.astype(bf16),
        "wfuseT": wfuseT_h.astype(bf16),
        "bfuse": np.asarray(inp["b_fuse"], np.float32).reshape(1, P2).astype(bf16),
    }


# ---------------------------------------------------------------- bass module
def _build_module():
    from contextlib import ExitStack
    import concourse.bass as bass  # noqa: F401
    import concourse.mybir as mybir
    import concourse.tile as tile
    from concourse import bacc

    dt = mybir.dt
    AX = mybir.AxisListType
    AF = mybir.ActivationFunctionType
    from concourse.tile_rust import add_dep_helper

    nc = bacc.Bacc("TRN2", target_bir_lowering=False, debug=False)

    # The xbar transpose's completion semaphore fires at issue, long before
    # its writes land.  Two mitigations (both required for HW correctness):
    #  - fences keep the DMA system quiet around each transpose so the
    #    landing lag stays bounded by the transpose's own transfer time;
    #  - gpsimd delay chains (below) gate consumers past that bounded lag.
    _dma_log = []
    _fence = [None]

    def dma(out, in_):
        inst = nc.sync.dma_start(out=out, in_=in_)
        if _fence[0] is not None:
            add_dep_helper(inst.ins, _fence[0].ins, sync=True,
                           reason="dma waits on transpose fence")
        _dma_log.append(inst)
        return inst

    def dmat(out, in_):
        inst = nc.sync.dma_start_transpose(out=out, in_=in_)
        for d in _dma_log:
            add_dep_helper(inst.ins, d.ins, sync=True,
                           reason="transpose waits on prior DMAs")
        if _fence[0] is not None:
            add_dep_helper(inst.ins, _fence[0].ins, sync=True,
                           reason="transpose waits on previous transpose")
        _dma_log.clear()
        _fence[0] = inst
        return inst

    import os
    reps = int(os.environ.get("CCK_REPS", "1"))
    debug_taps = bool(int(os.environ.get("CCK_DEBUG_TAPS", "0")))
    tap_d = {}
    if debug_taps:
        tap_d["tap_fT"] = nc.dram_tensor(
            "tap_fT", [2, 128, 2 * NB * 128], dt.bfloat16, kind="ExternalOutput").ap()
        tap_d["tap_oT"] = nc.dram_tensor(
            "tap_oT", [2, 128, 4 * NB * 128], dt.bfloat16, kind="ExternalOutput").ap()
        tap_d["tap_oc"] = nc.dram_tensor(
            "tap_oc", [NPC, 2, 128, NB * 128], dt.bfloat16, kind="ExternalOutput").ap()

    x_d = nc.dram_tensor("x", [NPC, C, HW], dt.float32r, kind="ExternalInput").ap()
    wconvT_d = nc.dram_tensor("wconvT", [128, 4 * CM], dt.float32r, kind="ExternalInput").ap()
    bconv_d = nc.dram_tensor("bconv", [CM, 1], dt.float32, kind="ExternalInput").ap()
    T_d = nc.dram_tensor("Tmat", [128, 84 * 128], dt.bfloat16, kind="ExternalInput").ap()
    wfuseT_d = nc.dram_tensor("wfuseT", [CM, 256], dt.bfloat16, kind="ExternalInput").ap()
    bfuse_d = nc.dram_tensor("bfuse", [1, P2], dt.bfloat16, kind="ExternalInput").ap()
    y_d = nc.dram_tensor("y", [NPC, 2 * P2, HW], dt.float32, kind="ExternalOutput").ap()

    with tile.TileContext(nc) as tc, ExitStack() as ctx:
        consts = ctx.enter_context(tc.tile_pool(name="consts", bufs=1))
        xpool = ctx.enter_context(tc.tile_pool(name="xp", bufs=4))
        big = ctx.enter_context(tc.tile_pool(name="big", bufs=1))
        ysp = ctx.enter_context(tc.tile_pool(name="ysp", bufs=2))
        small = ctx.enter_context(tc.tile_pool(name="small", bufs=8))
        ps_c1 = ctx.enter_context(tc.tile_pool(name="psc1", bufs=2, space="PSUM"))
        ps_sw = ctx.enter_context(tc.tile_pool(name="pssw", bufs=2, space="PSUM"))
        ps_fu = ctx.enter_context(tc.tile_pool(name="psfu", bufs=2, space="PSUM"))
        drp = ctx.enter_context(tc.tile_pool(name="drp", bufs=2, space="DRAM"))

        def dmat_via_dram(out, in_, tag):
            # SBUF-source xbar transposes corrupt on HW even when fenced;
            # stage through DRAM (the production-tested source).
            stage = drp.tile([128, in_.free_size()], in_.dtype, tag=tag)
            dma(out=stage, in_=in_)
            return dmat(out=out, in_=stage)

        # ---- constants to SBUF
        wconvT = consts.tile([128, 4, CM], dt.float32r)
        dma(out=wconvT, in_=wconvT_d)
        Tm = consts.tile([128, 84, 128], dt.bfloat16)
        dma(out=Tm, in_=T_d)
        wfuseT = consts.tile([CM, 2, 128], dt.bfloat16)
        dma(out=wfuseT, in_=wfuseT_d)
        bfuse = consts.tile([1, P2], dt.bfloat16)
        dma(out=bfuse, in_=bfuse_d)
        bconv = consts.tile([CM, 1], dt.float32)
        dma(out=bconv, in_=bconv_d)
        ones = consts.tile([1, FW], dt.bfloat16)
        nc.vector.memset(ones, 1.0)

        # ---- big single-buffer tensors
        # ffg: [c, map(0=f,1=f*g), j, s];  fT_all/oT/oc_all as commented
        ffg = big.tile([128, 2, 2, SP], dt.bfloat16)
        fT_all = big.tile([128, 2, 2, NB, 128], dt.bfloat16)   # [s_loc, map, j, b, c]
        oT = big.tile([128, 2, 2, NB, 128], dt.bfloat16)       # [s_loc, map, j, b, c]
        oc_all = big.tile([128, 2, 2, NB, 128], dt.bfloat16)   # [c, map, j, b, s_loc]
        for mp in range(2):
            for j in range(2):
                nc.gpsimd.memset(ffg[:, mp, j, HW:SP], 0.0)

        # xbar write-landing delay chain (see dma/dmat comments above)
        dumA = big.tile([128, 1280], dt.bfloat16)
        dumB = big.tile([128, 1280], dt.bfloat16)
        nc.gpsimd.memset(dumA, 0.0)

        def delay_chain(after_inst, hops):
            prev = after_inst
            for k in range(hops):
                s, d_ = (dumA, dumB) if k % 2 == 0 else (dumB, dumA)
                op = nc.gpsimd.tensor_copy(d_, s)
                add_dep_helper(op.ins, prev.ins, sync=True,
                               reason="xbar write-landing delay")
                prev = op
            return prev

        def tslice(kid, ph, pos):
            return Tm[:, kid * 21 + ph * 3 + pos, :]

        for rep in range(reps):
          for pair in range(2):
            for j in range(2):
                n = 2 * pair + j
                xc = []
                for kc in range(4):
                    xt = xpool.tile([128, HW], dt.float32r, tag="x")
                    dma(out=xt, in_=x_d[n, kc * 128:(kc + 1) * 128, :])
                    xc.append(xt)
                gpart = small.tile([128, 8], dt.float32)
                for e in range(8):
                    ps = ps_c1.tile([128, EW], dt.float32)
                    for kc in range(4):
                        nc.tensor.matmul(ps, wconvT[:, kc, :],
                                         xc[kc][:, e * EW:(e + 1) * EW],
                                         start=(kc == 0), stop=(kc == 3))
                    nc.vector.reduce_sum(gpart[:, e:e + 1], ps, axis=AX.X)
                    nc.scalar.activation(ffg[:, 0, j, e * EW:(e + 1) * EW], ps,
                                         AF.Relu, bias=bconv[:, 0:1], scale=1.0)
                gsum = small.tile([128, 1], dt.float32)
                nc.vector.reduce_sum(gsum, gpart, axis=AX.X)
                g = small.tile([128, 1], dt.float32)
                nc.scalar.activation(g, gsum, AF.Relu, bias=bconv[:, 0:1],
                                     scale=1.0 / HW)
                nc.vector.tensor_scalar_mul(ffg[:, 1, j, :HW], ffg[:, 0, j, :HW],
                                            g[:, 0:1])
            # ONE pair-wide transpose: [c, (map, j, s)] -> [s_loc, (map, j, b), c]
            t_i = dmat_via_dram(fT_all.rearrange("p a b c d -> p (a b c) d"),
                                ffg.rearrange("p a b c -> p (a b c)"), "fstage")
            gate_f = delay_chain(t_i, 10)

            # ---- conv sweeps for this pair (both samples in the free dim)
            for b in range(NB):
                ph = b % PH
                for m in range(2):
                    kidA, kidB = (0, 1) if m == 0 else (2, 3)
                    ps = ps_sw.tile([128, 2, 128], dt.float32, tag=f"sw{m}")
                    mms = []
                    for pos, d in ((0, -1), (1, 0), (2, 1)):
                        bi = b + d
                        if 0 <= bi < NB:
                            mms.append((kidA, 1, pos, bi))   # A-kernel on f*g
                            mms.append((kidB, 0, pos, bi))   # B-kernel on f
                    for i, (kid, mp, pos, bi) in enumerate(mms):
                        mm = nc.tensor.matmul(ps, tslice(kid, ph, pos),
                                              fT_all[:, mp, :, bi, :],
                                              start=(i == 0),
                                              stop=(i == len(mms) - 1))
                        add_dep_helper(mm.ins, gate_f.ins, sync=True,
                                       reason="sweep waits xbar landing")
                    if (b + m) % 2 == 0:
                        nc.scalar.activation(oT[:, m, :, b, :], ps, AF.Copy)
                    else:
                        nc.vector.tensor_copy(oT[:, m, :, b, :], ps)

            # ONE back transpose for the whole pair -> chan-major oc_all
            toc_i = dmat_via_dram(oc_all.rearrange("p a b c d -> p (a b c) d"),
                                  oT.rearrange("p a b c d -> p (a b c d)"),
                                  "ostage")
            gate_oc = delay_chain(toc_i, 10)

            # ---- fuse per (sample, map)
            for j in range(2):
                n = 2 * pair + j
                for m in range(2):
                    ocf = oc_all[:, m, j].rearrange("p a b -> p (a b)")
                    for ch in range(2):
                        row0 = m * 256 + ch * 128
                        for half in range(2):
                            yst = ysp.tile([128, 1600], dt.float32, tag="yst")
                            for fb in range(4):
                                col = half * 1600 + fb * FW
                                ps = ps_fu.tile([128, FW], dt.float32)
                                mmf = nc.tensor.matmul(ps, wfuseT[:, ch, :],
                                                       ocf[:, col:col + FW],
                                                       start=True, stop=False)
                                add_dep_helper(mmf.ins, gate_oc.ins, sync=True,
                                               reason="fuse waits xbar landing")
                                nc.tensor.matmul(ps, bfuse[:, ch * 128:(ch + 1) * 128],
                                                 ones[:, :],
                                                 start=False, stop=True)
                                dst = yst[:, fb * FW:(fb + 1) * FW]
                                if (fb + half + ch + m + n) % 2 == 0:
                                    nc.scalar.activation(dst, ps, AF.Copy)
                                else:
                                    nc.vector.tensor_copy(dst, ps)
                            wout = 1600 if half == 0 else HW - 1600
                            dma(out=y_d[n, row0:row0 + 128,
                                        half * 1600:half * 1600 + wout],
                                in_=yst[:, :wout])

    nc.compile()
    return nc


def _get_module():
    if "nc" not in _CACHE:
        _CACHE["nc"] = _build_module()
    return _CACHE["nc"]


# ---------------------------------------------------------------- entry point
def _run(inputs, trace=False, **kwargs):
    from concourse.bass_utils import run_bass_kernel_spmd

    nc = _get_module()
    consts = _host_consts(inputs)
    x = np.asarray(inputs["x"], np.float32).reshape(N, C, HW)
    in_maps = []
    for i in range(NCORES):
        m = dict(consts)
        m["x"] = np.ascontiguousarray(x[i * NPC:(i + 1) * NPC])
        in_maps.append(m)
    return run_bass_kernel_spmd(nc, in_maps, core_ids=list(range(NCORES)),
                                trace=trace, **kwargs)


def kernel(**inputs):
    res = _run(inputs)
    y = np.concatenate([r["y"] for r in res.results], axis=0)
    return y.reshape(N, 2 * P2, H, W).astype(np.float32)


if __name__ == "__main__":
    rng = np.random.default_rng(0)
    demo = {
        "x": rng.standard_normal((N, C, H, W), np.float32),
        "W_conv": 0.05 * rng.standard_normal((CM, C), np.float32),
        "b_conv": 0.05 * rng.standard_normal(CM).astype(np.float32),
        "wk": 0.05 * rng.standard_normal(25).astype(np.float32),
        "bk": 0.05 * rng.standard_normal(25).astype(np.float32),
        "wck": np.float32(0.03), "bck": np.float32(0.01),
        "wk2": 0.05 * rng.standard_normal(9).astype(np.float32),
        "bk2": 0.05 * rng.standard_normal(9).astype(np.float32),
        "wck2": np.float32(0.02), "bck2": np.float32(-0.01),
        "W_fuse": 0.05 * rng.standard_normal((P2, CM), np.float32),
        "b_fuse": 0.05 * rng.standard_normal(P2).astype(np.float32),
    }
    out = kernel(**demo)
    print(out.shape, out.dtype)



# revision 6
# speedup vs baseline: 7.2493x; 7.2493x over previous
"""Trainium2 Bass kernel for nn_DIDAModule (dense_cnn) — transpose-free v2.

Math: the per-sample "dynamic" depthwise kernels are affine in the channel
gate g:  kern1 = g*A1 + B1  with  A1 = wk*wck, B1 = bk*wck + bck  (5x5) and
A2 = wk2*wck2, B2 = bk2*wck2 + bck2 (3x3, dilation 2).  Per-channel scaling
commutes with the (channel-shared) depthwise convs, so with SA = conv_A(f),
SB = conv_B(f):   o = g * SA + SB   per branch, and
    y = [W_fuse @ o1 ; W_fuse @ o2]   (+ b_fuse added on the host).

Layout trick (no transposes anywhere):
  - conv1 runs "spatial-major": lhsT = x block [c,128s] (stationary),
    rhs = W_conv.T chunk [c,128o] -> psum [s, o] = f.T block.  The conv bias
    is a 1-row ones matmul into the same psum.  relu on the copy-out.
  - the depthwise convs are banded matmuls over spatial: lhsT = fT block
    [s_in, c], rhs = two adjacent band matrices [s_in, 2*128] (A1|A2 or
    B1|B2, both branches batched 256-wide) accumulating psum [c, 2br, m]
    == channel-major output, directly consumable by the fuse matmul.
  - combine o = g*SA + SB is one fused vector op (scalar_tensor_tensor)
    with g as a per-partition scalar.
  - fuse: lhsT = W_fuse.T chunk [c, o], rhs = oc [c, 512s] -> psum [o, s].
    b_fuse is added on the host after the gather (a bias matmul costs as
    much as the main matmul; host add is free w.r.t. HW time).

Sharding: data-parallel over batch N across the 8 cores (4 samples each),
weights replicated.  x is cast to bf16 on the host (halves input DMA).
"""

import numpy as np

# ---------------------------------------------------------------- dims
N, C, H, W = 32, 512, 56, 56
CM, K1, K2, P2 = 128, 5, 3, 256
HW = H * W            # 3136
NB = 25               # ceil(3136/128) blocks of 128 (last has 64 valid)
PH = 7                # phase classes (128 mod 56 period)
NCORES = 8
NPC = N // NCORES     # samples per core

_CACHE = {}


# ---------------------------------------------------------------- host prep
def _build_T(K2d, dil):
    """Banded conv matrices T[phase, pos, k_in, m_out] for flat 128-blocks."""
    kh = K2d.shape[0]
    r = (kh - 1) // 2 * dil
    T = np.zeros((PH, 3, 128, 128), np.float32)
    for p in range(PH):
        bref = 7 + p              # interior reference block of this phase
        for pos, d in enumerate((-1, 0, 1)):
            for m in range(128):
                s_out = bref * 128 + m
                ro, wo = divmod(s_out, W)
                for k in range(128):
                    s_in = (bref + d) * 128 + k
                    ri, wi = divmod(s_in, W)
                    di, dj = ri - ro, wi - wo
                    if (abs(di) <= r and abs(dj) <= r
                            and di % dil == 0 and dj % dil == 0):
                        T[p, pos, k, m] = K2d[di // dil + (kh - 1) // 2,
                                              dj // dil + (kh - 1) // 2]
    return T


def _host_consts(inp):
    import ml_dtypes
    bf16 = ml_dtypes.bfloat16
    W_conv = np.asarray(inp["W_conv"], np.float32)     # [CM, C]
    W_fuse = np.asarray(inp["W_fuse"], np.float32)     # [P2, CM]
    A1 = (np.asarray(inp["wk"]) * float(inp["wck"])).reshape(K1, K1)
    B1 = (np.asarray(inp["bk"]) * float(inp["wck"]) + float(inp["bck"])).reshape(K1, K1)
    A2 = (np.asarray(inp["wk2"]) * float(inp["wck2"])).reshape(K2, K2)
    B2 = (np.asarray(inp["bk2"]) * float(inp["wck2"]) + float(inp["bck2"])).reshape(K2, K2)
    # T layout: [k_in(128part), (ph, pos, kid4), m_out(128)], kid order
    # (A1, A2, B1, B2) so each (ph,pos) half is a 256-wide rhs slice.
    T = np.stack([_build_T(A1, 1), _build_T(A2, 2),
                  _build_T(B1, 1), _build_T(B2, 2)])        # [4,7,3,128,128]
    T_h = np.ascontiguousarray(T.transpose(3, 1, 2, 0, 4)).reshape(128, 84 * 128)
    # conv1 rhs chunks: [c_local(128part), chunk(4), o(128)] = W_conv.T chunks
    wconvT_h = np.ascontiguousarray(
        W_conv.T.reshape(4, 128, CM).transpose(1, 0, 2)).reshape(128, 4 * CM)
    # fuse lhsT chunks: [c(128part), chunk(2), o_local(128)]
    wfuseT_h = np.ascontiguousarray(
        W_fuse.T.reshape(CM, 2, 128)).reshape(CM, 256)
    return {
        "wconvT": wconvT_h.astype(bf16),
        "bconv": np.asarray(inp["b_conv"], np.float32).reshape(CM, 1),
        "bconvr": np.asarray(inp["b_conv"], np.float32).reshape(1, CM).astype(bf16),
        "Tmat": T_h.astype(bf16),
        "wfuseT": wfuseT_h.astype(bf16),
    }


# ---------------------------------------------------------------- bass module
def _build_module():
    from contextlib import ExitStack
    import concourse.bass as bass  # noqa: F401
    import concourse.mybir as mybir
    import concourse.tile as tile
    from concourse import bacc

    dt = mybir.dt
    AX = mybir.AxisListType
    AF = mybir.ActivationFunctionType
    OP = mybir.AluOpType

    nc = bacc.Bacc("TRN2", target_bir_lowering=False, debug=False)

    import os
    reps = int(os.environ.get("CCK_REPS", "1"))

    x_d = nc.dram_tensor("x", [NPC, C, HW], dt.bfloat16, kind="ExternalInput").ap()
    wconvT_d = nc.dram_tensor("wconvT", [128, 4 * CM], dt.bfloat16, kind="ExternalInput").ap()
    bconv_d = nc.dram_tensor("bconv", [CM, 1], dt.float32, kind="ExternalInput").ap()
    bconvr_d = nc.dram_tensor("bconvr", [1, CM], dt.bfloat16, kind="ExternalInput").ap()
    T_d = nc.dram_tensor("Tmat", [128, 84 * 128], dt.bfloat16, kind="ExternalInput").ap()
    wfuseT_d = nc.dram_tensor("wfuseT", [CM, 256], dt.bfloat16, kind="ExternalInput").ap()
    y_d = nc.dram_tensor("y", [NPC, 2 * P2, HW], dt.float32, kind="ExternalOutput").ap()

    with tile.TileContext(nc) as tc, ExitStack() as ctx:
        consts = ctx.enter_context(tc.tile_pool(name="consts", bufs=1))
        xp = ctx.enter_context(tc.tile_pool(name="xp", bufs=2))
        fp = ctx.enter_context(tc.tile_pool(name="fp", bufs=2))
        ocp = ctx.enter_context(tc.tile_pool(name="ocp", bufs=2))
        ysp = ctx.enter_context(tc.tile_pool(name="ysp", bufs=4))
        small = ctx.enter_context(tc.tile_pool(name="small", bufs=4))
        ps_c1 = ctx.enter_context(tc.tile_pool(name="psc1", bufs=2, space="PSUM"))
        ps_sw = ctx.enter_context(tc.tile_pool(name="pssw", bufs=3, space="PSUM"))
        ps_fu = ctx.enter_context(tc.tile_pool(name="psfu", bufs=2, space="PSUM"))
        ps_g = ctx.enter_context(tc.tile_pool(name="psg", bufs=1, space="PSUM"))

        # ---- constants to SBUF
        wconvT = consts.tile([128, 4, CM], dt.bfloat16)
        nc.sync.dma_start(out=wconvT, in_=wconvT_d)
        Tm = consts.tile([128, 84, 128], dt.bfloat16)
        nc.sync.dma_start(out=Tm, in_=T_d)
        wfuseT = consts.tile([CM, 2, 128], dt.bfloat16)
        nc.sync.dma_start(out=wfuseT, in_=wfuseT_d)
        bconv = consts.tile([CM, 1], dt.float32)
        nc.sync.dma_start(out=bconv, in_=bconv_d)
        bconvr = consts.tile([1, CM], dt.bfloat16)
        nc.sync.dma_start(out=bconvr, in_=bconvr_d)
        ones1 = consts.tile([1, 128], dt.bfloat16)
        nc.vector.memset(ones1, 1.0)

        def tsl(ph, pos, half):
            i = (ph * 3 + pos) * 4 + 2 * half
            return Tm[:, i:i + 2, :]

        for rep in range(reps):
          for j in range(NPC):
            # ---- x load (4 chunks of [128, HW] bf16)
            xt = xp.tile([128, 4, HW], dt.bfloat16, tag="x")
            for kc in range(4):
                nc.sync.dma_start(out=xt[:, kc, :],
                                  in_=x_d[j, kc * 128:(kc + 1) * 128, :])

            # ---- channel gate g = relu(mean_s(x) @ W.T + b)  [c,1] fp32
            xm = small.tile([128, 4], dt.float32, tag="xm")
            for kc in range(4):
                nc.vector.reduce_sum(xm[:, kc:kc + 1], xt[:, kc, :], axis=AX.X)
            xmb = small.tile([128, 4], dt.bfloat16, tag="xmb")
            nc.scalar.activation(xmb, xm, AF.Copy)
            gps = ps_g.tile([CM, 1], dt.float32, tag="g")
            for kc in range(4):
                nc.tensor.matmul(gps, wconvT[:, kc, :], xmb[:, kc:kc + 1],
                                 start=(kc == 0), stop=(kc == 3))
            g = small.tile([CM, 1], dt.float32, tag="gsb")
            nc.scalar.activation(g, gps, AF.Relu, bias=bconv[:, 0:1],
                                 scale=1.0 / HW)

            # ---- conv1, spatial-major: fT[s, b, o] = relu(x.T @ W.T + b)
            fT = fp.tile([128, NB, 128], dt.bfloat16, tag="fT")
            nc.gpsimd.memset(fT[64:128, NB - 1, :], 0.0)
            for b in range(NB):
                w = 128 if b < NB - 1 else HW - 128 * (NB - 1)
                ps = ps_c1.tile([128, 128], dt.float32, tag="c1")
                nc.tensor.matmul(ps[:w], ones1[:, :w], bconvr,
                                 start=True, stop=False)
                for kc in range(4):
                    nc.tensor.matmul(ps[:w], xt[:, kc, b * 128:b * 128 + w],
                                     wconvT[:, kc, :],
                                     start=False, stop=(kc == 3))
                if b % 2 == 0:
                    nc.scalar.activation(fT[:w, b, :], ps[:w], AF.Relu)
                else:
                    nc.vector.tensor_scalar_max(fT[:w, b, :], ps[:w], 0.0)

            # ---- banded depthwise sweeps -> channel-major oc
            # P[bo] = [c, ab, br, m]: [:,0]=A-kernels on f, [:,1]=B-kernels.
            oc = ocp.tile([128, 2, NB * 128], dt.bfloat16, tag="oc")
            P = {}

            def touch(bo):
                if bo not in P:
                    P[bo] = ps_sw.tile([128, 2, 2, 128], dt.float32,
                                       tag="P", name="P")
                return P[bo]

            def retire(bo):
                t = P.pop(bo)
                tA = small.tile([128, 2, 128], dt.bfloat16, tag="tA")
                nc.scalar.activation(tA, t[:, 0], AF.Copy, scale=g[:, 0:1])
                nc.vector.tensor_add(oc[:, :, bo * 128:(bo + 1) * 128],
                                     tA, t[:, 1])

            for bi in range(NB):
                for half in (0, 1):
                    for dd in (-1, 0, 1):
                        bo = bi - dd
                        if not (0 <= bo < NB):
                            continue
                        first = bi == max(bo - 1, 0)
                        last = bi == min(bo + 1, NB - 1)
                        nc.tensor.matmul(touch(bo)[:, half], fT[:, bi, :],
                                         tsl(bo % PH, dd + 1, half),
                                         start=first, stop=last)
                if bi >= 1:
                    retire(bi - 1)
            retire(NB - 1)

            # ---- fuse: y[o, s] = W_fuse @ oc  (bias on host)
            for br in range(2):
                for ch in range(2):
                    row0 = (br * 2 + ch) * 128
                    for t in range(7):
                        wt = 512 if t < 6 else HW - 6 * 512
                        ps = ps_fu.tile([128, 512], dt.float32, tag="fu")
                        nc.tensor.matmul(ps[:, :wt], wfuseT[:, ch, :],
                                         oc[:, br, t * 512:t * 512 + wt],
                                         start=True, stop=True)
                        yst = ysp.tile([128, 512], dt.float32, tag="yst")
                        if (br + ch + t) % 2 == 0:
                            nc.scalar.activation(yst[:, :wt], ps[:, :wt], AF.Copy)
                        else:
                            nc.vector.tensor_copy(yst[:, :wt], ps[:, :wt])
                        nc.sync.dma_start(
                            out=y_d[j, row0:row0 + 128, t * 512:t * 512 + wt],
                            in_=yst[:, :wt])

    nc.compile()
    return nc


def _get_module():
    if "nc" not in _CACHE:
        _CACHE["nc"] = _build_module()
    return _CACHE["nc"]


# ---------------------------------------------------------------- entry point
def _run(inputs, trace=False, **kwargs):
    import ml_dtypes
    from concourse.bass_utils import run_bass_kernel_spmd

    nc = _get_module()
    consts = _host_consts(inputs)
    x = np.asarray(inputs["x"], np.float32).reshape(N, C, HW).astype(ml_dtypes.bfloat16)
    in_maps = []
    for i in range(NCORES):
        m = dict(consts)
        m["x"] = np.ascontiguousarray(x[i * NPC:(i + 1) * NPC])
        in_maps.append(m)
    return run_bass_kernel_spmd(nc, in_maps, core_ids=list(range(NCORES)),
                                trace=trace, **kwargs)


def _finish(inputs, res):
    """Gather per-core outputs, add b_fuse (host-side), reshape to full."""
    y = np.concatenate([r["y"] for r in res.results], axis=0)  # [N, 512, HW]
    bf = np.asarray(inputs["b_fuse"], np.float32)
    y += np.concatenate([bf, bf])[None, :, None]
    return y.reshape(N, 2 * P2, H, W).astype(np.float32)


def kernel(**inputs):
    return _finish(inputs, _run(inputs))


if __name__ == "__main__":
    rng = np.random.default_rng(0)
    demo = {
        "x": rng.standard_normal((N, C, H, W), np.float32),
        "W_conv": 0.05 * rng.standard_normal((CM, C)).astype(np.float32),
        "b_conv": 0.05 * rng.standard_normal(CM).astype(np.float32),
        "wk": 0.05 * rng.standard_normal(25).astype(np.float32),
        "bk": 0.05 * rng.standard_normal(25).astype(np.float32),
        "wck": np.float32(0.03), "bck": np.float32(0.01),
        "wk2": 0.05 * rng.standard_normal(9).astype(np.float32),
        "bk2": 0.05 * rng.standard_normal(9).astype(np.float32),
        "wck2": np.float32(0.02), "bck2": np.float32(-0.01),
        "W_fuse": 0.05 * rng.standard_normal((P2, CM)).astype(np.float32),
        "b_fuse": 0.05 * rng.standard_normal(P2).astype(np.float32),
    }
    out = kernel(**demo)
    print(out.shape, out.dtype)
